# revision 3
# baseline (speedup 1.0000x reference)
"""Trainium2 Bass kernel for nn_PredictionPipeline (NMS detection).

Pipeline being implemented (see reference):
  sigmoid -> threshold 0.3 -> 3x3 stride-1 SAME maxpool -> peak mask
  -> first-`max_detections` nonzero in (c, y, x) order
  -> gather prob / size / side at those peaks.

Device strategy (memory-bound sweep over the 5 heatmap channels):
  * sigmoid is strictly monotonic, so all comparisons run on raw logits;
    the threshold becomes t = logit(0.3f).
  * A 3x3 stride-1 peak must equal the max of its aligned 2x2 block
    (every cell of the block is inside the peak's 3x3 window), so the
    device only computes 2x2 block maxes and emits the full-res
    candidate bitmap  cand = (v >= max(blockmax, t))  as uint8.
    This is a strict superset of the true peak set (~25% density).
  * 2x2 blocks never straddle core or partition boundaries, so spatial
    sharding along H needs no halo exchange at all.
  * Host verifies candidates (in raster order) against the exact 3x3
    window until `max_detections` peaks are confirmed - a few thousand
    cheap gathers - and computes the sigmoid gathers for the outputs.

Per-core traffic: 10.5 MB in + 2.6 MB out; three 1x DVE passes
(~1.75 cycles/pixel) keep VectorE near the DMA roofline.
"""

import os
import sys

import numpy as np

for _p in ("/opt/trn_rl_repo",):
    if os.path.isdir(_p) and _p not in sys.path:
        sys.path.append(_p)

# ---------------------------------------------------------------- constants
CH_HM = 5          # heatmap channels (peaks live here)
CH_SIDE = 5        # channel index of the side logit
CH_SIZE = 6        # channel index of the size logit
H = 2048
W = 2048
N_CORES = 8
RPC = H // N_CORES  # rows per core = 256
P = 128             # SBUF partitions; partition p holds row pair (2p, 2p+1)
FREE = 2 * W        # free-dim elements per partition = 4096 (two full rows)

THRESHOLD = 0.3
# raw-domain threshold: logit of the f32-rounded 0.3
T_CUT = float(
    np.log(np.float64(np.float32(THRESHOLD)) / (1.0 - np.float64(np.float32(THRESHOLD))))
)

_CACHE = {}


# ---------------------------------------------------------------- device
def _build_module():
    """Build + compile the per-core Bass module (identical on all cores)."""
    import concourse.bacc as bacc
    import concourse.mybir as mybir
    import concourse.tile as tile
    from concourse.bass_interp import get_hw_module

    f32 = mybir.dt.float32
    u8 = mybir.dt.uint8
    Op = mybir.AluOpType

    nc = bacc.Bacc("TRN2", target_bir_lowering=False, debug=False)
    x = nc.dram_tensor("x", [CH_HM, RPC, W], f32, kind="ExternalInput")
    y = nc.dram_tensor("y", [CH_HM, P, FREE], u8, kind="ExternalOutput")

    # DRAM view: channel ch, partition p holds rows (2p, 2p+1) -> each
    # partition's 16 KiB is contiguous in DRAM (max DMA efficiency).
    xr = x[:].rearrange("ch (p rr) w -> ch p (rr w)", p=P)

    with tile.TileContext(nc) as tc:
        with (
            tc.tile_pool(name="io", bufs=3) as io_pool,
            tc.tile_pool(name="tmp", bufs=2) as tmp_pool,
        ):
            for ch in range(CH_HM):
                v = io_pool.tile([P, FREE], f32, tag="v")
                nc.sync.dma_start(out=v[:], in_=xr[ch])

                # R[p, x] = max(row 2p, row 2p+1) at col x  (vertical pair max)
                R = tmp_pool.tile([P, W], f32, tag="R")
                nc.vector.tensor_tensor(R[:], v[:, 0:W], v[:, W:FREE], Op.max)

                # Bt[p, k] = max(blockmax(2x2), t)   (fused horizontal pair + thr)
                Bt = tmp_pool.tile([P, W // 2], f32, tag="Bt")
                rp = R[:].rearrange("p (k d) -> p k d", d=2)
                nc.vector.scalar_tensor_tensor(
                    Bt[:], rp[:, :, 0], T_CUT, rp[:, :, 1], Op.max, Op.max
                )

                # cand = (v >= Bt broadcast back to full res) as u8
                cand = io_pool.tile([P, FREE], u8, tag="cand")
                in0 = v[:].rearrange("p (rr k e) -> p rr k e", rr=2, e=2)
                btb = (
                    Bt[:]
                    .unsqueeze(1)
                    .unsqueeze(3)
                    .broadcast_to([P, 2, W // 2, 2])
                )
                outc = cand[:].rearrange("p (rr k e) -> p rr k e", rr=2, e=2)
                nc.vector.tensor_tensor(outc, in0, btb, Op.is_ge)

                nc.sync.dma_start(out=y[ch], in_=cand[:])

    nc.compile()
    nc.m = get_hw_module(nc.m)
    return nc


def _get_module():
    if "nc" not in _CACHE:
        _CACHE["nc"] = _build_module()
    return _CACHE["nc"]


def _run_device(feature, trace=False):
    """Shard along H, run the SPMD kernel, return the full candidate bitmap
    [5, H, W] u8 plus the BassKernelResults (for profiling)."""
    from concourse.bass_utils import run_bass_kernel_spmd

    nc = _get_module()
    hm = feature[0, :CH_HM]  # [5, H, W] float32
    in_maps = [
        {"x": np.ascontiguousarray(hm[:, k * RPC : (k + 1) * RPC, :])}
        for k in range(N_CORES)
    ]
    res = run_bass_kernel_spmd(
        nc, in_maps, core_ids=list(range(N_CORES)), trace=trace
    )
    cand = np.empty((CH_HM, H, W), np.uint8)
    for k in range(N_CORES):
        a = res.results[k]["y"]  # [5, 128, 4096]; partition p = row pair
        cand[:, k * RPC : (k + 1) * RPC, :] = a.reshape(CH_HM, RPC, W)
    return cand, res


# ---------------------------------------------------------------- host side
def _verify_candidates(hm, flat_idx, need):
    """Exact 3x3 peak check (raw-logit domain) for candidates in raster
    order; stop once `need` peaks are confirmed. Returns (confirmed flat
    indices, total_confirmed, exhausted)."""
    HW_ = H * W
    confirmed = []
    n_conf = 0
    pos = 0
    pad = None
    chunk = 16384
    while pos < len(flat_idx):
        idx = flat_idx[pos : pos + chunk]
        pos += chunk
        c = idx // HW_
        rem = idx - c * HW_
        yy = rem // W
        xx = rem - yy * W
        v = hm[c, yy, xx]
        if pad is None:
            pad = np.pad(hm, ((0, 0), (1, 1), (1, 1)), constant_values=-np.inf)
        m = np.full(v.shape, -np.inf, np.float32)
        for dy in range(3):
            for dx in range(3):
                np.maximum(m, pad[c, yy + dy, xx + dx], out=m)
        ok = (v > np.float32(T_CUT)) & (v >= m)
        n_new = int(ok.sum())
        if n_new:
            confirmed.append(idx[ok])
        n_conf += n_new
        if n_conf >= need:
            return np.concatenate(confirmed), n_conf, False
    if confirmed:
        return np.concatenate(confirmed), n_conf, True
    return np.empty(0, np.int64), 0, True


def _sigmoid_f32(a):
    return (1.0 / (1.0 + np.exp(-a.astype(np.float64)))).astype(np.float32)


def _decode(feature, cand, max_detections):
    K = int(max_detections)
    hm = np.ascontiguousarray(feature[0, :CH_HM])  # [5, H, W]
    flat_idx = np.flatnonzero(cand.reshape(-1))
    conf, n_conf, exhausted = _verify_candidates(hm, flat_idx, K)

    if exhausted:
        n_valid = n_conf  # verified every candidate -> exact total peak count
    else:
        n_valid = K  # >= K peaks exist; only the comparison with K matters

    sel = conf[:K]
    n = len(sel)
    HW_ = H * W
    c = np.zeros(K, np.int64)
    yy = np.zeros(K, np.int64)
    xx = np.zeros(K, np.int64)
    c[:n] = sel // HW_
    rem = sel - c[:n] * HW_
    yy[:n] = rem // W
    xx[:n] = rem - yy[:n] * W

    valid = (np.arange(K) < n_valid).astype(np.float32)

    probs_full = np.max(_sigmoid_f32(hm[:, yy, xx]), axis=0)  # [K]
    sizes_act = _sigmoid_f32(feature[0, CH_SIZE, yy, xx])
    side_act = _sigmoid_f32(feature[0, CH_SIDE, yy, xx])

    coords = np.stack([c, yy, xx], axis=1).astype(np.int32)
    probs = (probs_full * valid).astype(np.float32)
    size_width = (sizes_act * valid).astype(np.float32)
    size_height = size_width.copy()
    side_sel = (side_act * valid).astype(np.float32)
    return probs, coords, size_width, size_height, side_sel


def kernel(feature, max_detections):
    feature = np.asarray(feature, dtype=np.float32)
    assert feature.shape == (1, 7, H, W), feature.shape
    cand, _ = _run_device(feature, trace=False)
    return _decode(feature, cand, max_detections)
